# revision 3
# baseline (speedup 1.0000x reference)
"""Trainium2 Bass kernel for the Dinomaly anomaly head (ViTill fuse + bottleneck
MLP + 8 linear-attention decoder blocks + feature-map assembly).

Sharding: sequence-parallel over the 4096 (batch x token) positions across 8
cores (512 tokens each; core c owns batch c//2, token half c%2). Params are
replicated. The only cross-core dependency is the linear-attention KV/k-sum
statistic, which is summed over a batch's full 1024 tokens: each core computes
its partial [16,64,65] stat and a pair-wise AllReduce ([0,1],[2,3],...)
combines the two halves.

Layout: activations are feature-major [C, T] on-chip so per-feature params
broadcast along the free dim; LayerNorm stats (sums over C, a cross-partition
reduction) run on the PE via ones-vector matmuls, and per-token scalars are
broadcast across partitions via tiny PE matmuls.

Precision: matmuls in bf16 (fp32 PSUM accumulate), residual stream in fp32r
(fp32 with 11-bit mantissa, required so the LN-stat matmuls can consume it at
full PE rate), LN/score scalar math in fp32.
"""

import os

import ml_dtypes
import numpy as np

import concourse.bass as bass
import concourse.mybir as mybir
import concourse.tile as tile
from concourse import bacc
from concourse.bass_utils import run_bass_kernel_spmd

F32 = mybir.dt.float32
F32R = mybir.dt.float32r
BF16 = mybir.dt.bfloat16
AF = mybir.ActivationFunctionType
OP = mybir.AluOpType

# Model dims (hardcoded per the problem spec).
B, L, N, C = 4, 8, 1024, 1024
H, DHEAD = 16, 64
HID = 4 * C
D = 8
EPS = 1e-8

NCORES = 8
T = (B * N) // NCORES          # 512 tokens per core
KC = C // 128                  # 8 feature chunks
TQ = T // 128                  # 4 token tiles
MG = 4                         # matmul m-group size (PSUM banks per group)

DT = BF16                      # matmul compute dtype
DT_NP = ml_dtypes.bfloat16

REPLICA_GROUPS = [[0, 1], [2, 3], [4, 5], [6, 7]]

_ONEHOT = np.zeros((2, 128), dtype=np.float32)
_ONEHOT[0, 0:64] = 1.0
_ONEHOT[1, 64:128] = 1.0


def _round_f32r(x: np.ndarray) -> np.ndarray:
    """Round fp32 to fp32r (RNE to 11 mantissa bits, low 12 bits zero)."""
    x = np.ascontiguousarray(x, dtype=np.float32)
    u = x.view(np.uint32)
    lo = (u >> 12) & 1
    return ((u + 0x7FF + lo) & 0xFFFFF000).view(np.float32)


def _cast_w(w: np.ndarray) -> np.ndarray:
    if DT is BF16:
        return np.ascontiguousarray(w, dtype=np.float32).astype(DT_NP)
    return _round_f32r(w)


def _slab(dram_ap, k0, kn, n0, nn):
    """DRAM weight slice [k0:k0+kn*128, n0:n0+nn] -> [128, kn, nn] AP."""
    return dram_ap[k0 * 128:(k0 + kn) * 128, n0:n0 + nn].rearrange(
        "(kc p) n -> p kc n", p=128
    )


def build_nc(collectives=True):
    nc = bacc.Bacc("TRN2", target_bir_lowering=False, debug=False)

    en_d = nc.dram_tensor("en", [L, C, T], F32, kind="ExternalInput")
    qkvw_d = nc.dram_tensor("qkvw", [D, C, 3 * C], DT, kind="ExternalInput")
    projw_d = nc.dram_tensor("projw", [D, C, C], DT, kind="ExternalInput")
    fc1w_d = nc.dram_tensor("fc1w", [D, C, HID], DT, kind="ExternalInput")
    fc2w_d = nc.dram_tensor("fc2w", [D, HID, C], DT, kind="ExternalInput")
    bn1w_d = nc.dram_tensor("bn1w", [C, HID], DT, kind="ExternalInput")
    bn2w_d = nc.dram_tensor("bn2w", [HID, C], DT, kind="ExternalInput")
    onehot_d = nc.dram_tensor("onehot", [2, 128], F32, kind="ExternalInput")
    out_d = nc.dram_tensor("out", [4, C, T], F32, kind="ExternalOutput")

    with tile.TileContext(nc) as tc:
        with (
            tc.tile_pool(name="wcg", bufs=3) as wcg,        # weight slabs + en stream
            tc.tile_pool(name="abuf", bufs=1) as abuf,      # block activations
            tc.tile_pool(name="xhp", bufs=2) as xhp,        # normalized inputs
            tc.tile_pool(name="carryp", bufs=2) as carryp,  # residual stream
            tc.tile_pool(name="accp", bufs=1) as accp,      # de accumulators
            tc.tile_pool(name="temps", bufs=4) as temps,    # [128,512] f32 temps
            tc.tile_pool(name="smalls", bufs=1) as smalls,
            tc.tile_pool(name="consts", bufs=1) as consts,
            tc.tile_pool(name="psA", bufs=MG, space="PSUM") as psA,
            tc.tile_pool(name="psB", bufs=2, space="PSUM") as psB,
            tc.tile_pool(name="pskv", bufs=2, space="PSUM") as pskvp,
            tc.tile_pool(name="dram", bufs=2, space="DRAM") as dramp,
        ):
            # ---- constants ----
            # memset can't write f32r; stage in f32 and tensor_copy (a valid
            # f32r-rounding producer).
            cstf = consts.tile([128, 128], F32, name="cstf")
            nc.vector.memset(cstf, 1.0)
            ones1 = consts.tile([128, 1], F32R, name="ones1")
            nc.vector.tensor_copy(out=ones1, in_=cstf[:, 0:1])
            ones128 = consts.tile([1, 128], F32R, name="ones128")
            nc.vector.tensor_copy(out=ones128, in_=cstf[0:1, :])
            oh_f = consts.tile([2, 128], F32, name="oh_f")
            nc.sync.dma_start(out=oh_f, in_=onehot_d[:, :])
            onehot2 = consts.tile([2, 128], F32R, name="onehot2")
            nc.vector.tensor_copy(out=onehot2, in_=oh_f)
            zsrc = consts.tile([128, 16], F32, name="zsrc")
            nc.vector.memset(zsrc, 0.0)
            eps_t = consts.tile([1, 1], F32, name="eps_t")
            nc.vector.memset(eps_t, EPS)

            acc0 = accp.tile([128, KC, T], F32, name="acc0")
            acc1 = accp.tile([128, KC, T], F32, name="acc1")
            nc.vector.memset(acc0, 0.0)
            nc.vector.memset(acc1, 0.0)

            # ---- stage 0: en means, x0 ----
            # acc1 accumulates layers 0-3, acc0 layers 4-7 (halves of KC at a
            # time through 8KB wcg slots).
            for l in range(L):
                dst = acc1 if l < 4 else acc0
                for hf in range(2):
                    sl = wcg.tile([128, 4, T], F32, tag="wcg", name=f"en{l}{hf}")
                    nc.sync.dma_start(out=sl, in_=_slab(en_d[l], hf * 4, 4, 0, T))
                    for kq in range(4):
                        kc = hf * 4 + kq
                        nc.vector.tensor_tensor(
                            dst[:, kc, :], dst[:, kc, :], sl[:, kq, :], op=OP.add
                        )

            en0_st = carryp.tile([128, KC, T], F32, tag="carry", name="en0_st")
            nc.vector.tensor_scalar_mul(en0_st, acc1, 0.25)
            nc.sync.dma_start(out=_slab(out_d[0], 0, KC, 0, T), in_=en0_st)

            xsum = carryp.tile([128, KC, T], F32, tag="carry", name="xsum")
            nc.vector.tensor_tensor(xsum, acc1, acc0, op=OP.add)
            x0 = xhp.tile([128, KC, T], DT, tag="xh", name="x0")
            nc.vector.tensor_scalar_mul(x0, xsum, 0.125)

            en1_st = carryp.tile([128, KC, T], F32, tag="carry", name="en1_st")
            nc.vector.tensor_scalar_mul(en1_st, acc0, 0.25)
            nc.sync.dma_start(out=_slab(out_d[1], 0, KC, 0, T), in_=en1_st)

            nc.vector.memset(acc0, 0.0)
            nc.vector.memset(acc1, 0.0)

            # ---- helpers ----
            def mm_layer(w_dram, rhs_tile, kin, mtiles, out_cb, name):
                """out[m] = sum_k W[k, m].T @ rhs[k]; m-grouped, K-accumulated.

                w_dram: [kin*128, mtiles*128] DRAM AP; rhs_tile [128, kin, T];
                out_cb(mi, psum_tile) consumes each finished [128, T] output.
                """
                n_groups = (mtiles + MG - 1) // MG
                for g in range(n_groups):
                    ms = [g * MG + i for i in range(MG) if g * MG + i < mtiles]
                    nsub = (kin + KC - 1) // KC
                    pss = {}
                    for m in ms:
                        pss[m] = psA.tile([128, T], F32, tag="big",
                                          name=f"{name}_ps{m}")
                    for sb in range(nsub):
                        k0, kn = sb * KC, min(KC, kin - sb * KC)
                        wt = wcg.tile([128, kn, len(ms) * 128], DT, tag="wcg",
                                      name=f"{name}_w{g}_{sb}")
                        nc.sync.dma_start(
                            out=wt,
                            in_=_slab(w_dram, k0, kn, ms[0] * 128, len(ms) * 128),
                        )
                        for kc in range(kn):
                            for j, m in enumerate(ms):
                                nc.tensor.matmul(
                                    pss[m],
                                    wt[:, kc, j * 128:(j + 1) * 128],
                                    rhs_tile[:, k0 + kc, :],
                                    start=(sb == 0 and kc == 0),
                                    stop=(sb == nsub - 1 and kc == kn - 1),
                                )
                    for m in ms:
                        out_cb(m, pss[m])

            def layer_norm(carry, xh, name):
                """xh = (carry - mean)/std, feature-major, stats over C."""
                ps_sum = psB.tile([1, T], F32, tag="aux", name=f"{name}_sum")
                ps_ssq = psB.tile([1, T], F32, tag="aux", name=f"{name}_ssq")
                for kc in range(KC):
                    sq = temps.tile([128, T], F32R, tag="tmp", name=f"{name}_sq{kc}")
                    nc.vector.tensor_tensor(sq, carry[:, kc, :], carry[:, kc, :],
                                            op=OP.mult)
                    nc.tensor.matmul(ps_sum, ones1, carry[:, kc, :],
                                     start=(kc == 0), stop=(kc == KC - 1))
                    nc.tensor.matmul(ps_ssq, ones1, sq,
                                     start=(kc == 0), stop=(kc == KC - 1))
                m_sb = smalls.tile([1, T], F32R, tag="m_sb", name=f"{name}_m")
                nc.vector.tensor_scalar_mul(m_sb, ps_sum, 1.0 / C)
                msq = temps.tile([1, T], F32, tag="tmp", name=f"{name}_msq")
                nc.vector.tensor_tensor(msq, m_sb, m_sb, op=OP.mult)
                var = temps.tile([1, T], F32, tag="tmp", name=f"{name}_var")
                nc.vector.tensor_scalar_mul(var, ps_ssq, 1.0 / C)
                nc.vector.tensor_tensor(var, var, msq, op=OP.subtract)
                std = temps.tile([1, T], F32, tag="tmp", name=f"{name}_std")
                nc.scalar.activation(out=std, in_=var, func=AF.Sqrt, bias=eps_t)
                rstd = smalls.tile([1, T], F32R, tag="rstd", name=f"{name}_rstd")
                with nc.allow_low_precision(reason="f32r feeds PE broadcast"):
                    nc.vector.reciprocal(out=rstd, in_=std)

                ps_mpl = psB.tile([128, T], F32, tag="aux", name=f"{name}_mpl")
                nc.tensor.matmul(ps_mpl, ones128, m_sb, start=True, stop=True)
                ps_rpl = psB.tile([128, T], F32, tag="aux", name=f"{name}_rpl")
                nc.tensor.matmul(ps_rpl, ones128, rstd, start=True, stop=True)
                for kc in range(KC):
                    t1 = temps.tile([128, T], F32, tag="tmp", name=f"{name}_c{kc}")
                    nc.vector.tensor_tensor(t1, carry[:, kc, :], ps_mpl,
                                            op=OP.subtract)
                    nc.vector.tensor_tensor(xh[:, kc, :], t1, ps_rpl, op=OP.mult)

            def elu1(ps_in, out_ap, name):
                """out = elu(x)+1 = exp(min(x,0)) + relu(x), from PSUM."""
                mn = temps.tile([128, T], F32, tag="tmp", name=f"{name}_mn")
                nc.vector.tensor_scalar_min(mn, ps_in, 0.0)
                e = temps.tile([128, T], F32, tag="tmp", name=f"{name}_e")
                nc.scalar.activation(out=e, in_=mn, func=AF.Exp)
                r = temps.tile([128, T], F32, tag="tmp", name=f"{name}_r")
                nc.scalar.activation(out=r, in_=ps_in, func=AF.Relu)
                nc.vector.tensor_tensor(out_ap, e, r, op=OP.add)

            # ---- stage 1: bottleneck MLP ----
            hbuf = abuf.tile([128, HID // 128, T], DT, tag="h", name="bn_h")

            def bn_gelu(m, ps):
                nc.scalar.activation(out=hbuf[:, m, :], in_=ps, func=AF.Gelu)

            mm_layer(bn1w_d[:, :], x0, KC, HID // 128, bn_gelu, "bnf1")

            carry = carryp.tile([128, KC, T], F32R, tag="carry", name="carry_bn")

            def bn_out(m, ps):
                nc.vector.tensor_copy(out=carry[:, m, :], in_=ps)

            mm_layer(bn2w_d[:, :], hbuf, HID // 128, KC, bn_out, "bnf2")

            # ---- stage 2: decoder blocks ----
            for d in range(D):
                xh = xhp.tile([128, KC, T], DT, tag="xh", name=f"b{d}_xh")
                layer_norm(carry, xh, f"b{d}ln1")

                # k, v token-major: out[t, feat] tiles [128, 512]
                kT = abuf.tile([128, TQ, C], DT, tag="kT", name=f"b{d}_kT")
                vA = abuf.tile([128, TQ, H, DHEAD + 1], DT, tag="vA",
                               name=f"b{d}_vA")
                nc.vector.memset(vA[:, :, :, DHEAD:DHEAD + 1], 1.0)
                for cg in range(2):       # two 512-col groups of k feats
                    wt = wcg.tile([128, KC, 512], DT, tag="wcg",
                                  name=f"b{d}_wk{cg}")
                    nc.sync.dma_start(
                        out=wt, in_=_slab(qkvw_d[d], 0, KC, C + cg * 512, 512))
                    for tt in range(TQ):
                        ps = psA.tile([128, 512], F32, tag="big",
                                      name=f"b{d}_psk{cg}{tt}")
                        for kc in range(KC):
                            nc.tensor.matmul(
                                ps, xh[:, kc, tt * 128:(tt + 1) * 128],
                                wt[:, kc, :],
                                start=(kc == 0), stop=(kc == KC - 1))
                        elu1(ps, kT[:, tt, cg * 512:(cg + 1) * 512], f"b{d}ek{cg}{tt}")
                for cg in range(2):       # v
                    wt = wcg.tile([128, KC, 512], DT, tag="wcg",
                                  name=f"b{d}_wv{cg}")
                    nc.sync.dma_start(
                        out=wt, in_=_slab(qkvw_d[d], 0, KC, 2 * C + cg * 512, 512))
                    for tt in range(TQ):
                        ps = psA.tile([128, 512], F32, tag="big",
                                      name=f"b{d}_psv{cg}{tt}")
                        for kc in range(KC):
                            nc.tensor.matmul(
                                ps, xh[:, kc, tt * 128:(tt + 1) * 128],
                                wt[:, kc, :],
                                start=(kc == 0), stop=(kc == KC - 1))
                        nc.vector.tensor_copy(
                            out=vA[:, tt, cg * 8:(cg + 1) * 8, 0:DHEAD],
                            in_=ps.rearrange("p (h e) -> p h e", h=8))

                # kv partial: per head [64, 65], packed in pairs on partitions
                kvps = [pskvp.tile([128, 4, 128], F32, tag="kv",
                                   name=f"b{d}_kv{i}") for i in range(2)]
                for h in range(H):
                    j, p = h // 2, 64 * (h % 2)
                    ps = kvps[j // 4]
                    for tt in range(TQ):
                        nc.tensor.matmul(
                            ps[p:p + 64, j % 4, 0:DHEAD + 1],
                            kT[:, tt, h * 64:h * 64 + 64],
                            vA[:, tt, h, :],
                            start=(tt == 0), stop=(tt == TQ - 1))
                kvp = smalls.tile([128, 8, DHEAD + 1], F32, tag="kvp",
                                  name=f"b{d}_kvp")
                nc.vector.tensor_copy(out=kvp[:, 0:4, :],
                                      in_=kvps[0][:, :, 0:DHEAD + 1])
                nc.vector.tensor_copy(out=kvp[:, 4:8, :],
                                      in_=kvps[1][:, :, 0:DHEAD + 1])

                ar_in = dramp.tile([128, 8 * (DHEAD + 1)], F32, tag="arin",
                                   name=f"b{d}_arin")
                ar_out = dramp.tile([128, 8 * (DHEAD + 1)], F32, tag="arout",
                                    name=f"b{d}_arout")
                nc.sync.dma_start(out=ar_in,
                                  in_=kvp.rearrange("p a b -> p (a b)"))
                if collectives:
                    nc.gpsimd.collective_compute(
                        "AllReduce", OP.add,
                        ins=[ar_in.opt()], outs=[ar_out.opt()],
                        replica_groups=REPLICA_GROUPS)
                else:
                    nc.sync.dma_start(out=ar_out, in_=ar_in)
                kvf = smalls.tile([128, 8, DHEAD + 1], F32, tag="kvf",
                                  name=f"b{d}_kvf")
                nc.sync.dma_start(out=kvf.rearrange("p a b -> p (a b)"),
                                  in_=ar_out)
                kv_sb = smalls.tile([128, 8, DHEAD + 1], DT, tag="kvsb",
                                    name=f"b{d}_kvsb")
                nc.vector.tensor_copy(out=kv_sb, in_=kvf)

                # q (feature-major) while the AllReduce is in flight
                qe = abuf.tile([128, KC, T], F32R, tag="qe", name=f"b{d}_qe")

                def q_elu(m, ps, _d=d):
                    elu1(ps, qe[:, m, :], f"b{_d}eq{m}")

                mm_layer(qkvw_d[d][:, 0:C], xh, KC, KC, q_elu, f"b{d}q")

                # block-diag ksum for den; den per chunk -> z -> qz
                bd = smalls.tile([128, KC, 2], F32R, tag="bd", name=f"b{d}_bd")
                nc.vector.tensor_copy(out=bd.rearrange("p a b -> p (a b)"),
                                      in_=zsrc)
                for h in range(H):
                    c, e = h // 2, h % 2
                    nc.vector.tensor_copy(
                        out=bd[64 * e:64 * e + 64, c, e:e + 1],
                        in_=kvf[64 * e:64 * e + 64, c, DHEAD:DHEAD + 1])
                qz = abuf.tile([128, KC, T], DT, tag="qz", name=f"b{d}_qz")
                for c in range(KC):
                    psden = psB.tile([2, T], F32, tag="aux", name=f"b{d}_den{c}")
                    nc.tensor.matmul(psden, bd[:, c, :], qe[:, c, :],
                                     start=True, stop=True)
                    z2 = temps.tile([2, T], F32R, tag="tmp", name=f"b{d}_z{c}")
                    with nc.allow_low_precision(reason="f32r feeds PE broadcast"):
                        nc.vector.reciprocal(out=z2, in_=psden)
                    pszb = psB.tile([128, T], F32, tag="aux", name=f"b{d}_zb{c}")
                    nc.tensor.matmul(pszb, onehot2, z2, start=True, stop=True)
                    nc.vector.tensor_tensor(qz[:, c, :], qe[:, c, :], pszb,
                                            op=OP.mult)

                # attention out per head -> attn_sb (feature-major)
                attn_sb = abuf.tile([128, KC, T], DT, tag="attn",
                                    name=f"b{d}_attn")
                for j in range(KC):
                    psat = psA.tile([128, T], F32, tag="big", name=f"b{d}_at{j}")
                    for e in range(2):
                        p = 64 * e
                        nc.tensor.matmul(
                            psat[p:p + 64, :],
                            kv_sb[p:p + 64, j, 0:DHEAD],
                            qz[p:p + 64, j, :],
                            start=True, stop=True)
                    nc.vector.tensor_copy(out=attn_sb[:, j, :], in_=psat)

                # proj + residual
                carry2 = carryp.tile([128, KC, T], F32R, tag="carry",
                                     name=f"b{d}_carry2")

                def proj_out(m, ps, _c=carry, _c2=carry2):
                    nc.vector.tensor_tensor(_c2[:, m, :], _c[:, m, :], ps,
                                            op=OP.add)

                mm_layer(projw_d[d][:, :], attn_sb, KC, KC, proj_out, f"b{d}pr")

                # mlp
                xh2 = xhp.tile([128, KC, T], DT, tag="xh", name=f"b{d}_xh2")
                layer_norm(carry2, xh2, f"b{d}ln2")
                hb = abuf.tile([128, HID // 128, T], DT, tag="h", name=f"b{d}_h")

                def mlp_gelu(m, ps, _h=hb):
                    nc.scalar.activation(out=_h[:, m, :], in_=ps, func=AF.Gelu)

                mm_layer(fc1w_d[d][:, :], xh2, KC, HID // 128, mlp_gelu,
                         f"b{d}f1")

                carry3 = carryp.tile([128, KC, T], F32R, tag="carry",
                                     name=f"b{d}_carry3")
                accd = acc1 if d < 4 else acc0

                def mlp_out(m, ps, _c2=carry2, _c3=carry3, _a=accd):
                    nc.vector.tensor_tensor(_c3[:, m, :], _c2[:, m, :], ps,
                                            op=OP.add)
                    nc.vector.tensor_tensor(_a[:, m, :], _a[:, m, :],
                                            _c3[:, m, :], op=OP.add)

                mm_layer(fc2w_d[d][:, :], hb, HID // 128, KC, mlp_out, f"b{d}f2")
                carry = carry3

            # ---- stage 3: de maps ----
            de0 = carryp.tile([128, KC, T], F32, tag="carry", name="de0_st")
            nc.vector.tensor_scalar_mul(de0, acc0, 0.25)
            nc.sync.dma_start(out=_slab(out_d[2], 0, KC, 0, T), in_=de0)
            de1 = carryp.tile([128, KC, T], F32, tag="carry", name="de1_st")
            nc.vector.tensor_scalar_mul(de1, acc1, 0.25)
            nc.sync.dma_start(out=_slab(out_d[3], 0, KC, 0, T), in_=de1)

    nc.compile()
    return nc


_NC_CACHE = None


def kernel(**inputs) -> np.ndarray:
    global _NC_CACHE
    en_feats = np.asarray(inputs["en_feats"], dtype=np.float32)

    # Fold LayerNorm affine params into the following matmul weights (host-side
    # preprocessing of replicated params). Biases in this module are all zero;
    # verify and skip them on device.
    for bname in ("bn_fc1_b", "bn_fc2_b", "qkv_b", "proj_b", "mlp_fc1_b",
                  "mlp_fc2_b", "ln1_b", "ln2_b"):
        assert np.abs(np.asarray(inputs[bname])).max() == 0.0, bname
    ln1_w = np.asarray(inputs["ln1_w"], dtype=np.float32)
    ln2_w = np.asarray(inputs["ln2_w"], dtype=np.float32)
    qkvw = np.asarray(inputs["qkv_w"], dtype=np.float32) * ln1_w[:, :, None]
    fc1w = np.asarray(inputs["mlp_fc1_w"], dtype=np.float32) * ln2_w[:, :, None]

    wmap = {
        "qkvw": _cast_w(qkvw),
        "projw": _cast_w(np.asarray(inputs["proj_w"], dtype=np.float32)),
        "fc1w": _cast_w(fc1w),
        "fc2w": _cast_w(np.asarray(inputs["mlp_fc2_w"], dtype=np.float32)),
        "bn1w": _cast_w(np.asarray(inputs["bn_fc1_w"], dtype=np.float32)),
        "bn2w": _cast_w(np.asarray(inputs["bn_fc2_w"], dtype=np.float32)),
    }

    in_maps = []
    for c in range(NCORES):
        b, hf = c // 2, c % 2
        sl = en_feats[b, :, hf * T:(hf + 1) * T, :]          # [L, T, C]
        en_c = np.ascontiguousarray(sl.transpose(0, 2, 1))   # [L, C, T]
        in_maps.append({"en": en_c, **wmap, "onehot": _ONEHOT})

    if _NC_CACHE is None:
        _NC_CACHE = build_nc()
    nc = _NC_CACHE

    trace = os.environ.get("BASS_KERNEL_TRACE", "0") == "1"
    res = run_bass_kernel_spmd(nc, in_maps, core_ids=list(range(NCORES)),
                               trace=trace)
    if trace and res.exec_time_ns is not None:
        print(f"HW exec time: {res.exec_time_ns} ns")
        if res.instructions_and_trace is not None:
            print(f"trace: {res.instructions_and_trace[1]}")

    out = np.empty((4, B, C, N), dtype=np.float32)
    for c in range(NCORES):
        b, hf = c // 2, c % 2
        out[:, b, :, hf * T:(hf + 1) * T] = res.results[c]["out"]
    return out.reshape(4, B, C, 32, 32)



# revision 22
# speedup vs baseline: 1.0881x; 1.0881x over previous
"""Trainium2 Bass kernel for the Dinomaly anomaly head (ViTill fuse + bottleneck
MLP + 8 linear-attention decoder blocks + feature-map assembly).

Sharding: sequence-parallel over the 4096 (batch x token) positions across 8
cores (512 tokens each; core c owns batch c//2, token half c%2). Params are
replicated. The only cross-core dependency is the linear-attention KV/k-sum
statistic, which is summed over a batch's full 1024 tokens: each core computes
its partial [16,64,65] stat and a pair-wise AllReduce ([0,1],[2,3],...)
combines the two halves.

Layout: activations are feature-major [C, T] on-chip so per-feature params
broadcast along the free dim; LayerNorm stats (sums over C, a cross-partition
reduction) run on the PE via ones-vector matmuls, and per-token scalars are
broadcast across partitions via tiny PE matmuls.

Precision: matmuls in bf16 (fp32 PSUM accumulate), residual stream in fp32r
(fp32 with 11-bit mantissa, required so the LN-stat matmuls can consume it at
full PE rate), LN/score scalar math in fp32.

Engine split: PE does all matmuls (including LN stat/broadcast); DVE handles
critical-path elementwise (LN normalize, elu min/add, residual adds); the
scalar engine does exp/relu/gelu/sqrt plus x^2 for LN stats and PSUM-drain
copies; gpsimd (Pool) takes SBUF-side non-critical work (de accumulators,
en/de map scaling, kv-stat unpack) since it has no PSUM port.
"""

import os

import ml_dtypes
import numpy as np

import concourse.bass as bass
import concourse.mybir as mybir
import concourse.tile as tile
from concourse import bacc
from concourse.bass_utils import run_bass_kernel_spmd

F32 = mybir.dt.float32
F32R = mybir.dt.float32r
BF16 = mybir.dt.bfloat16
AF = mybir.ActivationFunctionType
OP = mybir.AluOpType
AX = mybir.AxisListType

# Model dims (hardcoded per the problem spec).
B, L, N, C = 4, 8, 1024, 1024
H, DHEAD = 16, 64
HID = 4 * C
D = 8
EPS = 1e-8

NCORES = 8
T = (B * N) // NCORES          # 512 tokens per core
KC = C // 128                  # 8 feature chunks
TQ = T // 128                  # 4 token tiles
MG = 3                         # matmul m-group size (PSUM banks per group)

DT = BF16                      # matmul compute dtype
DT_NP = ml_dtypes.bfloat16

REPLICA_GROUPS = [[0, 1], [2, 3], [4, 5], [6, 7]]

_ONEHOT = np.zeros((2, 128), dtype=np.float32)
_ONEHOT[0, 0:64] = 1.0
_ONEHOT[1, 64:128] = 1.0


def _round_f32r(x: np.ndarray) -> np.ndarray:
    """Round fp32 to fp32r (RNE to 11 mantissa bits, low 12 bits zero)."""
    x = np.ascontiguousarray(x, dtype=np.float32)
    u = x.view(np.uint32)
    lo = (u >> 12) & 1
    return ((u + 0x7FF + lo) & 0xFFFFF000).view(np.float32)


def _cast_w(w: np.ndarray) -> np.ndarray:
    if DT is BF16:
        return np.ascontiguousarray(w, dtype=np.float32).astype(DT_NP)
    return _round_f32r(w)


def _slab(dram_ap, k0, kn, n0, nn):
    """DRAM weight slice [k0:k0+kn*128, n0:n0+nn] -> [128, kn, nn] AP."""
    return dram_ap[k0 * 128:(k0 + kn) * 128, n0:n0 + nn].rearrange(
        "(kc p) n -> p kc n", p=128
    )


def build_nc(collectives=True, repeat=1):
    nc = bacc.Bacc("TRN2", target_bir_lowering=False, debug=False)

    en_d = nc.dram_tensor("en", [L, C, T], F32, kind="ExternalInput")
    qkvw_d = nc.dram_tensor("qkvw", [D, C, 3 * C], DT, kind="ExternalInput")
    projw_d = nc.dram_tensor("projw", [D, C, C], DT, kind="ExternalInput")
    fc1w_d = nc.dram_tensor("fc1w", [D, C, HID], DT, kind="ExternalInput")
    fc2w_d = nc.dram_tensor("fc2w", [D, HID, C], DT, kind="ExternalInput")
    bn1w_d = nc.dram_tensor("bn1w", [C, HID], DT, kind="ExternalInput")
    bn2w_d = nc.dram_tensor("bn2w", [HID, C], DT, kind="ExternalInput")
    onehot_d = nc.dram_tensor("onehot", [2, 128], F32, kind="ExternalInput")
    out_d = nc.dram_tensor("out", [4, C, T], F32, kind="ExternalOutput")

    with tile.TileContext(nc) as tc:
        with (
            tc.tile_pool(name="wcg", bufs=3) as wcg,        # weight slabs + en stream
            tc.tile_pool(name="abuf", bufs=1) as abuf,      # block activations
            tc.tile_pool(name="xhp", bufs=2) as xhp,        # normalized inputs
            tc.tile_pool(name="carryp", bufs=2) as carryp,  # residual stream
            tc.tile_pool(name="accp", bufs=1) as accp,      # de accumulators
            tc.tile_pool(name="temps", bufs=4) as temps,    # [128,512] f32 temps
            tc.tile_pool(name="t16", bufs=6) as t16p,       # [128,512] bf16 temps
            tc.tile_pool(name="outs", bufs=2) as outsp,     # en/de map staging
            tc.tile_pool(name="smalls", bufs=1) as smalls,
            tc.tile_pool(name="consts", bufs=1) as consts,
            tc.tile_pool(name="psA", bufs=2 * MG, space="PSUM") as psA,
            tc.tile_pool(name="psB", bufs=2, space="PSUM") as psB,
            tc.tile_pool(name="dram", bufs=2, space="DRAM") as dramp,
        ):
            # ---- constants ----
            # memset can't write f32r; stage in f32 and tensor_copy (a valid
            # f32r-rounding producer).
            cstf = consts.tile([128, 128], F32, name="cstf")
            nc.vector.memset(cstf, 1.0)
            ones1 = consts.tile([128, 1], F32R, name="ones1")
            nc.vector.tensor_copy(out=ones1, in_=cstf[:, 0:1])
            ones128 = consts.tile([1, 128], F32R, name="ones128")
            nc.vector.tensor_copy(out=ones128, in_=cstf[0:1, :])
            # sqrt(C) row: folds the 1/C variance scale into the rstd
            # broadcast (rpl = sqrt(C)/sqrt(ssq - sum*m + C*eps)).
            rootc = consts.tile([1, 128], F32R, name="rootc")
            nc.vector.tensor_scalar_mul(rootc, cstf[0:1, :], float(C) ** 0.5)
            epsc = consts.tile([1, 1], F32, name="epsc")
            nc.vector.memset(epsc, float(C) * EPS)
            oh_f = consts.tile([2, 128], F32, name="oh_f")
            nc.sync.dma_start(out=oh_f, in_=onehot_d[:, :])
            onehot2 = consts.tile([2, 128], F32R, name="onehot2")
            nc.vector.tensor_copy(out=onehot2, in_=oh_f)

            def one_pass():
                acc0 = accp.tile([128, KC, T], F32, name="acc0")
                acc1 = accp.tile([128, KC, T], F32, name="acc1")

                # ---- stage 0: en means, x0 (chunk-streamed so the
                # bottleneck MLP can start on chunk 0 immediately).
                # bnf1's first m-group (6 PSUM banks, per-chunk weight
                # slabs interleaved into the DMA stream) trickles on the PE
                # while the en stream lands; the rest of bnf1 runs dense.
                G0 = 2 * MG
                x0 = xhp.tile([128, KC, T], DT, tag="xh", name="x0")
                hbuf = abuf.tile([128, HID // 128, T], DT, tag="h", name="bn_h")
                g0ps = [psA.tile([128, T], F32, tag="big", name=f"bnf1_g0p{m}")
                        for m in range(G0)]
                for kc in range(KC):
                    for half, dst in ((0, acc1), (1, acc0)):
                        sl = wcg.tile([128, 4, T], F32, tag="wcg",
                                      name=f"en{kc}{half}")
                        nc.sync.dma_start(
                            out=sl,
                            in_=en_d[half * 4:half * 4 + 4,
                                     kc * 128:(kc + 1) * 128, :].rearrange(
                                         "l p t -> p l t"))
                        nc.vector.tensor_reduce(
                            out=dst[:, kc, :],
                            in_=sl.rearrange("p l t -> p t l"),
                            axis=AX.X, op=OP.add)
                    t0 = temps.tile([128, T], F32, tag="tmp", name=f"x0s{kc}")
                    nc.vector.tensor_tensor(t0, acc0[:, kc, :], acc1[:, kc, :],
                                            op=OP.add)
                    nc.vector.tensor_scalar_mul(x0[:, kc, :], t0, 0.125)
                    wt = wcg.tile([128, 1, G0 * 128], DT, tag="wcg",
                                  name=f"bnf1_g0w{kc}")
                    nc.sync.dma_start(out=wt,
                                      in_=_slab(bn1w_d, kc, 1, 0, G0 * 128))
                    for m in range(G0):
                        nc.tensor.matmul(
                            g0ps[m], wt[:, 0, m * 128:(m + 1) * 128],
                            x0[:, kc, :],
                            start=(kc == 0), stop=(kc == KC - 1))

                # ---- helpers ----
                def mm_layer(w_dram, rhs_tile, kin, mtiles, out_cb, name,
                             stat_cb=None):
                    """out[m] = sum_k W[k, m].T @ rhs[k]; m-grouped, K-accum.

                    w_dram: [kin*128, mtiles*128] DRAM AP; rhs_tile
                    [128, kin, T]; out_cb(mi, psum_tile) consumes each
                    finished [128, T] output. stat_cb(mi) may emit PE work
                    that reads out_cb's result; it is lagged one group so
                    the PE never waits on the drain.
                    """
                    n_groups = (mtiles + MG - 1) // MG
                    prev_stat = []
                    for g in range(n_groups):
                        ms = [g * MG + i for i in range(MG) if g * MG + i < mtiles]
                        nsub = (kin + KC - 1) // KC
                        pss = {}
                        for m in ms:
                            pss[m] = psA.tile([128, T], F32, tag="big",
                                              name=f"{name}_ps{m}")
                        for sb in range(nsub):
                            k0, kn = sb * KC, min(KC, kin - sb * KC)
                            wt = wcg.tile([128, kn, len(ms) * 128], DT,
                                          tag="wcg", name=f"{name}_w{g}_{sb}")
                            nc.sync.dma_start(
                                out=wt,
                                in_=_slab(w_dram, k0, kn, ms[0] * 128,
                                          len(ms) * 128),
                            )
                            for kc in range(kn):
                                for j, m in enumerate(ms):
                                    nc.tensor.matmul(
                                        pss[m],
                                        wt[:, kc, j * 128:(j + 1) * 128],
                                        rhs_tile[:, k0 + kc, :],
                                        start=(sb == 0 and kc == 0),
                                        stop=(sb == nsub - 1 and kc == kn - 1),
                                    )
                        for m in prev_stat:
                            stat_cb(m)
                        for m in ms:
                            out_cb(m, pss[m])
                        if stat_cb is not None:
                            prev_stat = ms
                    for m in prev_stat:
                        stat_cb(m)

                # LayerNorm: stats accumulate per chunk (as the producer
                # emits each carry chunk), so only the short scalar chain
                # remains serialized at the block boundary. sum and sum-sq
                # pack into one [2, T] PSUM tile (two interleaved
                # accumulation groups on rows 0/1).
                def ln_stats(name):
                    ps_sum = psB.tile([1, T], F32, tag="aux", name=f"{name}_sum")
                    ps_ssq = psB.tile([1, T], F32, tag="aux", name=f"{name}_ssq")
                    return ps_sum, ps_ssq

                def ln_chunk(st, src, kc, name):
                    ps_sum, ps_ssq = st
                    sq = temps.tile([128, T], F32R, tag="tmp",
                                    name=f"{name}_sq{kc}")
                    nc.scalar.activation(out=sq, in_=src, func=AF.Square)
                    nc.tensor.matmul(ps_sum, ones1, src,
                                     start=(kc == 0), stop=(kc == KC - 1))
                    nc.tensor.matmul(ps_ssq, ones1, sq,
                                     start=(kc == 0), stop=(kc == KC - 1))

                def ln_finish(st, carry, xh, name):
                    # rstd' = 1/sqrt(ssq - sum*m + C*eps); the missing
                    # sqrt(C) rides on the rpl broadcast (rootc).
                    ps_sum, ps_ssq = st
                    m_sb = smalls.tile([1, T], F32R, tag="m_sb", name=f"{name}_m")
                    nc.vector.tensor_scalar_mul(m_sb, ps_sum, 1.0 / C)
                    s2 = temps.tile([1, T], F32, tag="tmp", name=f"{name}_s2")
                    nc.vector.tensor_tensor(s2, ps_sum, m_sb, op=OP.mult)
                    w = temps.tile([1, T], F32, tag="tmp", name=f"{name}_w")
                    nc.vector.tensor_tensor(w, ps_ssq, s2, op=OP.subtract)
                    sd = temps.tile([1, T], F32, tag="tmp", name=f"{name}_sd")
                    nc.scalar.activation(out=sd, in_=w, func=AF.Sqrt,
                                         bias=epsc)
                    rstd = smalls.tile([1, T], F32R, tag="rstd",
                                       name=f"{name}_rstd")
                    with nc.allow_low_precision(reason="f32r feeds PE bcast"):
                        nc.vector.reciprocal(out=rstd, in_=sd)
                    ps_mpl = psB.tile([128, T], F32, tag="aux", name=f"{name}_mpl")
                    nc.tensor.matmul(ps_mpl, ones128, m_sb,
                                     start=True, stop=True)
                    ps_rpl = psB.tile([128, T], F32, tag="aux", name=f"{name}_rpl")
                    nc.tensor.matmul(ps_rpl, rootc, rstd, start=True, stop=True)
                    for kc in range(KC):
                        t1 = temps.tile([128, T], F32, tag="tmp",
                                        name=f"{name}_c{kc}")
                        nc.vector.tensor_tensor(t1, carry[:, kc, :], ps_mpl,
                                                op=OP.subtract)
                        nc.vector.tensor_tensor(xh[:, kc, :], t1, ps_rpl,
                                                op=OP.mult)

                def elu1(ps_in, out_ap, name):
                    """out = elu(x)+1 = exp(min(x,0)) + relu(x), from PSUM.

                    bf16 temps keep the final add in the DVE 2x path."""
                    mn = t16p.tile([128, T], BF16, tag="t16", name=f"{name}_mn")
                    nc.vector.tensor_scalar_min(mn, ps_in, 0.0)
                    e = t16p.tile([128, T], BF16, tag="t16", name=f"{name}_e")
                    nc.scalar.activation(out=e, in_=mn, func=AF.Exp)
                    r = t16p.tile([128, T], BF16, tag="t16", name=f"{name}_r")
                    nc.scalar.activation(out=r, in_=ps_in, func=AF.Relu)
                    nc.vector.tensor_tensor(out_ap, e, r, op=OP.add)

                # ---- stage 1: bottleneck MLP ----
                def bn_gelu(m, ps):
                    nc.scalar.activation(out=hbuf[:, m, :], in_=ps, func=AF.Gelu)

                for m in range(G0):
                    bn_gelu(m, g0ps[m])
                mm_layer(bn1w_d[:, G0 * 128:], x0, KC, HID // 128 - G0,
                         lambda m, ps: bn_gelu(m + G0, ps), "bnf1")

                # en map staging late: keeps the out DMAs off the stage-0
                # DMA stream (acc0/acc1 stay valid until blocks 0/4).
                for kc in range(KC):
                    for i, acc in ((0, acc1), (1, acc0)):
                        st = outsp.tile([128, T], F32, tag="outst",
                                        name=f"en{i}st{kc}")
                        nc.gpsimd.tensor_scalar_mul(st, acc[:, kc, :], 0.25)
                        nc.sync.dma_start(out=_slab(out_d[i], kc, 1, 0, T),
                                          in_=st)

                carry = carryp.tile([128, KC, T], F32R, tag="carry",
                                    name="carry_bn")
                stats = ln_stats("b0ln1")

                def bn_out(m, ps, _c=carry):
                    nc.vector.tensor_copy(out=_c[:, m, :], in_=ps)

                def bn_stat(m, _st=stats, _c=carry):
                    ln_chunk(_st, _c[:, m, :], m, "b0ln1")

                mm_layer(bn2w_d[:, :], hbuf, HID // 128, KC, bn_out, "bnf2",
                         stat_cb=bn_stat)

                # ---- stage 2: decoder blocks ----
                for d in range(D):
                    xh = xhp.tile([128, KC, T], DT, tag="xh", name=f"b{d}_xh")
                    ln_finish(stats, carry, xh, f"b{d}ln1")

                    # k, v token-major: out[t, feat] tiles [128, 512]
                    kT = abuf.tile([128, TQ, C], DT, tag="kT", name=f"b{d}_kT")
                    vA = abuf.tile([128, TQ, H, DHEAD + 1], DT, tag="vA",
                                   name=f"b{d}_vA")
                    nc.vector.memset(vA[:, :, :, DHEAD:DHEAD + 1], 1.0)
                    for cg in range(2):       # two 512-col groups of k feats
                        wt = wcg.tile([128, KC, 512], DT, tag="wcg",
                                      name=f"b{d}_wk{cg}")
                        nc.sync.dma_start(
                            out=wt, in_=_slab(qkvw_d[d], 0, KC, C + cg * 512, 512))
                        for tt in range(TQ):
                            ps = psA.tile([128, 512], F32, tag="big",
                                          name=f"b{d}_psk{cg}{tt}")
                            for kc in range(KC):
                                nc.tensor.matmul(
                                    ps, xh[:, kc, tt * 128:(tt + 1) * 128],
                                    wt[:, kc, :],
                                    start=(kc == 0), stop=(kc == KC - 1))
                            elu1(ps, kT[:, tt, cg * 512:(cg + 1) * 512],
                                 f"b{d}ek{cg}{tt}")
                    for cg in range(2):       # v
                        wt = wcg.tile([128, KC, 512], DT, tag="wcg",
                                      name=f"b{d}_wv{cg}")
                        nc.sync.dma_start(
                            out=wt, in_=_slab(qkvw_d[d], 0, KC, 2 * C + cg * 512, 512))
                        for tt in range(TQ):
                            ps = psA.tile([128, 512], F32, tag="big",
                                          name=f"b{d}_psv{cg}{tt}")
                            for kc in range(KC):
                                nc.tensor.matmul(
                                    ps, xh[:, kc, tt * 128:(tt + 1) * 128],
                                    wt[:, kc, :],
                                    start=(kc == 0), stop=(kc == KC - 1))
                            nc.scalar.copy(
                                out=vA[:, tt, cg * 8:(cg + 1) * 8, 0:DHEAD],
                                in_=ps.rearrange("p (h e) -> p h e", h=8))

                    # kv partial: per head [64, 65], packed in pairs on
                    # partitions
                    kvps = [psB.tile([128, 4, 128], F32, tag="aux",
                                     name=f"b{d}_kv{i}") for i in range(2)]
                    for h in range(H):
                        j, p = h // 2, 64 * (h % 2)
                        ps = kvps[j // 4]
                        for tt in range(TQ):
                            nc.tensor.matmul(
                                ps[p:p + 64, j % 4, 0:DHEAD + 1],
                                kT[:, tt, h * 64:h * 64 + 64],
                                vA[:, tt, h, :],
                                start=(tt == 0), stop=(tt == TQ - 1))
                    kvp = smalls.tile([128, 8, DHEAD + 1], F32, tag="kvp",
                                      name=f"b{d}_kvp")
                    nc.scalar.copy(out=kvp[:, 0:4, :],
                                   in_=kvps[0][:, :, 0:DHEAD + 1])
                    nc.scalar.copy(out=kvp[:, 4:8, :],
                                   in_=kvps[1][:, :, 0:DHEAD + 1])

                    ar_in = dramp.tile([128, 8 * (DHEAD + 1)], F32, tag="arin",
                                       name=f"b{d}_arin")
                    ar_out = dramp.tile([128, 8 * (DHEAD + 1)], F32,
                                        tag="arout", name=f"b{d}_arout")
                    nc.sync.dma_start(out=ar_in,
                                      in_=kvp.rearrange("p a b -> p (a b)"))
                    if collectives:
                        nc.gpsimd.collective_compute(
                            "AllReduce", OP.add,
                            ins=[ar_in.opt()], outs=[ar_out.opt()],
                            replica_groups=REPLICA_GROUPS)
                    else:
                        nc.sync.dma_start(out=ar_out, in_=ar_in)
                    kvf = smalls.tile([128, 8, DHEAD + 1], F32, tag="kvf",
                                      name=f"b{d}_kvf")
                    nc.sync.dma_start(out=kvf.rearrange("p a b -> p (a b)"),
                                      in_=ar_out)
                    kv_sb = smalls.tile([128, 8, DHEAD + 1], DT, tag="kvsb",
                                        name=f"b{d}_kvsb")
                    nc.gpsimd.tensor_copy(out=kv_sb, in_=kvf)

                    # q (feature-major) while the AllReduce is in flight
                    qe = abuf.tile([128, KC, T], DT, tag="qe", name=f"b{d}_qe")

                    def q_elu(m, ps, _d=d):
                        elu1(ps, qe[:, m, :], f"b{_d}eq{m}")

                    mm_layer(qkvw_d[d][:, 0:C], xh, KC, KC, q_elu, f"b{d}q")

                    # block-diag ksum for den; den per chunk -> z -> qz
                    bd = smalls.tile([128, KC, 2], DT, tag="bd", name=f"b{d}_bd")
                    nc.gpsimd.memset(bd, 0.0)
                    for e in range(2):
                        p = 64 * e
                        nc.gpsimd.tensor_copy(
                            out=bd[p:p + 64, :, e:e + 1],
                            in_=kvf[p:p + 64, :, DHEAD:DHEAD + 1])
                    qz = abuf.tile([128, KC, T], DT, tag="qz", name=f"b{d}_qz")
                    for c in range(KC):
                        psden = psB.tile([2, T], F32, tag="aux",
                                         name=f"b{d}_den{c}")
                        nc.tensor.matmul(psden, bd[:, c, :], qe[:, c, :],
                                         start=True, stop=True)
                        z2 = temps.tile([2, T], F32R, tag="tmp", name=f"b{d}_z{c}")
                        with nc.allow_low_precision(reason="f32r PE bcast"):
                            nc.vector.reciprocal(out=z2, in_=psden)
                        pszb = psB.tile([128, T], F32, tag="aux",
                                        name=f"b{d}_zb{c}")
                        nc.tensor.matmul(pszb, onehot2, z2, start=True, stop=True)
                        nc.vector.tensor_tensor(qz[:, c, :], qe[:, c, :], pszb,
                                                op=OP.mult)

                    # attention out per head -> attn_sb (feature-major)
                    attn_sb = abuf.tile([128, KC, T], DT, tag="attn",
                                        name=f"b{d}_attn")
                    for j in range(KC):
                        psat = psA.tile([128, T], F32, tag="big",
                                        name=f"b{d}_at{j}")
                        for e in range(2):
                            p = 64 * e
                            nc.tensor.matmul(
                                psat[p:p + 64, :],
                                kv_sb[p:p + 64, j, 0:DHEAD],
                                qz[p:p + 64, j, :],
                                start=True, stop=True)
                        nc.scalar.copy(out=attn_sb[:, j, :], in_=psat)

                    # proj + residual; ln2 stats accumulate as chunks land
                    carry2 = carryp.tile([128, KC, T], F32R, tag="carry",
                                         name=f"b{d}_carry2")
                    stats2 = ln_stats(f"b{d}ln2")

                    def proj_out(m, ps, _c=carry, _c2=carry2):
                        nc.vector.tensor_tensor(_c2[:, m, :], _c[:, m, :], ps,
                                                op=OP.add)

                    def proj_stat(m, _c2=carry2, _st=stats2, _d=d):
                        ln_chunk(_st, _c2[:, m, :], m, f"b{_d}ln2")

                    mm_layer(projw_d[d][:, :], attn_sb, KC, KC, proj_out,
                             f"b{d}pr", stat_cb=proj_stat)

                    # mlp
                    xh2 = xhp.tile([128, KC, T], DT, tag="xh", name=f"b{d}_xh2")
                    ln_finish(stats2, carry2, xh2, f"b{d}ln2")
                    hb = abuf.tile([128, HID // 128, T], DT, tag="h",
                                   name=f"b{d}_h")

                    def mlp_gelu(m, ps, _h=hb):
                        nc.scalar.activation(out=_h[:, m, :], in_=ps, func=AF.Gelu)

                    mm_layer(fc1w_d[d][:, :], xh2, KC, HID // 128, mlp_gelu,
                             f"b{d}f1")

                    carry3 = carryp.tile([128, KC, T], F32R, tag="carry",
                                         name=f"b{d}_carry3")
                    accd = acc1 if d < 4 else acc0
                    stats = ln_stats(f"b{d + 1}ln1") if d < D - 1 else None
                    # de1 = acc1/4 is final after block 3; de0 = acc0/4 after
                    # block 7 -> stream the map out as its chunks finalize.
                    emit = {3: (3, acc1), 7: (2, acc0)}.get(d)

                    def mlp_out(m, ps, _c2=carry2, _c3=carry3, _a=accd,
                                _em=emit, _first=(d == 0 or d == 4)):
                        nc.vector.tensor_tensor(_c3[:, m, :], _c2[:, m, :], ps,
                                                op=OP.add)
                        if _first:
                            nc.gpsimd.tensor_copy(out=_a[:, m, :],
                                                  in_=_c3[:, m, :])
                        else:
                            nc.gpsimd.tensor_tensor(_a[:, m, :], _a[:, m, :],
                                                    _c3[:, m, :], op=OP.add)
                        if _em is not None:
                            oi, acc = _em
                            st_t = outsp.tile([128, T], F32, tag="outst",
                                              name=f"de{oi}st{m}")
                            nc.gpsimd.tensor_scalar_mul(st_t, acc[:, m, :], 0.25)
                            nc.sync.dma_start(out=_slab(out_d[oi], m, 1, 0, T),
                                              in_=st_t)

                    def mlp_stat(m, _c3=carry3, _st=stats, _d=d):
                        if _st is not None:
                            ln_chunk(_st, _c3[:, m, :], m, f"b{_d + 1}ln1")

                    mm_layer(fc2w_d[d][:, :], hb, HID // 128, KC, mlp_out,
                             f"b{d}f2", stat_cb=mlp_stat if d < D - 1 else None)
                    carry = carry3

            for _rep in range(repeat):
                one_pass()

    nc.compile()
    return nc


_NC_CACHE = None


def kernel(**inputs) -> np.ndarray:
    global _NC_CACHE
    en_feats = np.asarray(inputs["en_feats"], dtype=np.float32)

    # Fold LayerNorm affine params into the following matmul weights (host-side
    # preprocessing of replicated params). Biases in this module are all zero;
    # verify and skip them on device.
    for bname in ("bn_fc1_b", "bn_fc2_b", "qkv_b", "proj_b", "mlp_fc1_b",
                  "mlp_fc2_b", "ln1_b", "ln2_b"):
        assert np.abs(np.asarray(inputs[bname])).max() == 0.0, bname
    ln1_w = np.asarray(inputs["ln1_w"], dtype=np.float32)
    ln2_w = np.asarray(inputs["ln2_w"], dtype=np.float32)
    qkvw = np.asarray(inputs["qkv_w"], dtype=np.float32) * ln1_w[:, :, None]
    fc1w = np.asarray(inputs["mlp_fc1_w"], dtype=np.float32) * ln2_w[:, :, None]

    wmap = {
        "qkvw": _cast_w(qkvw),
        "projw": _cast_w(np.asarray(inputs["proj_w"], dtype=np.float32)),
        "fc1w": _cast_w(fc1w),
        "fc2w": _cast_w(np.asarray(inputs["mlp_fc2_w"], dtype=np.float32)),
        "bn1w": _cast_w(np.asarray(inputs["bn_fc1_w"], dtype=np.float32)),
        "bn2w": _cast_w(np.asarray(inputs["bn_fc2_w"], dtype=np.float32)),
    }

    in_maps = []
    for c in range(NCORES):
        b, hf = c // 2, c % 2
        sl = en_feats[b, :, hf * T:(hf + 1) * T, :]          # [L, T, C]
        en_c = np.ascontiguousarray(sl.transpose(0, 2, 1))   # [L, C, T]
        in_maps.append({"en": en_c, **wmap, "onehot": _ONEHOT})

    if _NC_CACHE is None:
        _NC_CACHE = build_nc()
    nc = _NC_CACHE

    trace = os.environ.get("BASS_KERNEL_TRACE", "0") == "1"
    res = run_bass_kernel_spmd(nc, in_maps, core_ids=list(range(NCORES)),
                               trace=trace)
    if trace and res.exec_time_ns is not None:
        print(f"HW exec time: {res.exec_time_ns} ns")
        if res.instructions_and_trace is not None:
            print(f"trace: {res.instructions_and_trace[1]}")

    out = np.empty((4, B, C, N), dtype=np.float32)
    for c in range(NCORES):
        b, hf = c // 2, c % 2
        out[:, b, :, hf * T:(hf + 1) * T] = res.results[c]["out"]
    return out.reshape(4, B, C, 32, 32)
